# revision 2
# baseline (speedup 1.0000x reference)
"""VQ Euclidean-codebook kernel for Trainium2 (8 NeuronCores, data-parallel).

Parity-split argmax: PE emits per K-half an even-codeword score stream (PSUM)
and an odd stream (ACT-copied to SBUF); one custom DVE scan computes
max(se,so), running-max qualification, and accum MAX of
qual*(2*(slot+1)+(so>se)) -- the pair-resolved argmax index. Cross-half
combine: one indirect gather of both candidates' [e_row|c] from an embC
table, a fused dot-scan rescores them exactly, arithmetic select emits the
winning codebook row.

PE per tile: 8 fp16 matmuls (exact-x [x1;x2] split, C=128) + 8 fp8 DoubleRow
correction matmuls (e5m2 weights x e4m3 moving) adding x.(e-fp16(e)) and the
bias -0.5||e||^2 as six scaled e4m3 chunks on spare contraction rows.
"""

import numpy as np

import concourse.bass as bass
import concourse.bacc as bacc
import concourse.mybir as mybir
from concourse.tile import TileContext
from concourse.bass_utils import run_bass_kernel_spmd

from concourse import dve_ops
from concourse.dve_spec import (
    Spec, Src0, Src1, Zero, C1, AluOp, scan, eq, lower, Bin, Scan,
)
from concourse.dve_uop import DveOpSpec

P = 128
N_FULL = 131072
N_CORES = 8
N_LOC = N_FULL // N_CORES
K = 4096
D = 64
NT = N_LOC // P
F32 = mybir.dt.float32
F16 = mybir.dt.float16
E5 = mybir.dt.float8e5
E4 = mybir.dt.float8e4
I32 = mybir.dt.int32

SCALE = float(2.0 ** 11)
CHUNK_A = [2, 6, 10, 14, 18, 22]
CHUNK_W = [min(a, 14) for a in CHUNK_A]

_OP = "PARITY_ARGMAX_ANT"
_OP2 = "DOT_SCAN_ANT"


def _ref_parity(in0, in1, c0, c1, c2):
    se = np.asarray(in0, np.float32).reshape(in0.shape[0], -1)
    so = np.asarray(in1, np.float32).reshape(in1.shape[0], -1)
    v = np.maximum(se, so)
    r = np.maximum.accumulate(v, axis=1)
    qual = (v == r).astype(np.float32)
    n = v.shape[1]
    idx2 = ((np.arange(n, dtype=np.float32) + 1.0) * np.float32(c1))[None, :]
    isodd = (se < so).astype(np.float32)
    body = (qual * (idx2 + isodd)).astype(np.float32)
    acc = body.max(axis=1, keepdims=True)
    return body.reshape(in0.shape), acc


def _ref_dotscan(in0, in1, c0, c1, c2):
    prod = (np.asarray(in0, np.float32) * np.asarray(in1, np.float32))
    p2 = prod.reshape(prod.shape[0], -1).astype(np.float32)
    return np.cumsum(p2, axis=1, dtype=np.float32).reshape(in0.shape)


def _register(name, spec):
    for op in dve_ops.OPS:
        if op.name == name:
            return op
    row = dve_ops._CUSTOM_DVE_ROW_BASE + len(dve_ops.OPS)
    dve_ops._SUB_OPCODE_FOR_NAME[name] = row
    uops = lower(spec, ver="v3")
    sha = DveOpSpec(name=name, opcode=row, uops=uops, rd1_en=True).sha("v3")
    op = dve_ops.DveOp(name, spec, subdim=False, uops_sha={"v3": sha})
    dve_ops.OPS.append(op)
    dve_ops.CUSTOM_DVE_SPECS[name] = spec
    return op


def register_ops():
    v = Bin(AluOp.MAX, Src0, Src1)
    qual = eq(v, scan(AluOp.MAX, v))
    isodd = Bin(AluOp.IS_LT, Src0, Src1)
    idx2 = Scan(AluOp.ADD, C1, init=Zero)
    op1 = _register(_OP, Spec(body=qual * (idx2 + isodd), accum=AluOp.MAX,
                              reference=_ref_parity))
    op2 = _register(_OP2, Spec(body=scan(AluOp.ADD, Src0 * Src1),
                               reference=_ref_dotscan))
    return op1, op2


def build(r_iters: int = 1):
    argmax_op, dot_op = register_ops()
    nc = bacc.Bacc(num_devices=N_CORES)
    xT_in = nc.dram_tensor("xT", [D, N_LOC], F32, kind="ExternalInput")
    embT_in = nc.dram_tensor("embT", [D, K], F32, kind="ExternalInput")
    emb_in = nc.dram_tensor("embed", [K, D], F32, kind="ExternalInput")
    xrm_in = nc.dram_tensor("xrm", [N_LOC, D], F32, kind="ExternalInput")
    q_out = nc.dram_tensor("q", [N_LOC, D], F32, kind="ExternalOutput")
    embC = nc.dram_tensor("embC", [K, 68], F32, kind="Internal")

    KQ = K // 4      # columns per parity stream per half
    KHH = K // 4

    with TileContext(nc) as tc:
        with (
            tc.tile_pool(name="const", bufs=1) as cpool,
            tc.tile_pool(name="sodd", bufs=3) as sopool,
            tc.tile_pool(name="junk", bufs=2) as jpool,
            tc.tile_pool(name="small", bufs=4) as mpool,
            tc.tile_pool(name="gath", bufs=4) as gpool,
            tc.tile_pool(name="pse", bufs=2, space="PSUM") as pepool,
            tc.tile_pool(name="pso", bufs=2, space="PSUM") as popool,
        ):
            # ---------------- persistent tables ----------------
            xs = cpool.tile([P, N_LOC], F16)
            w8 = cpool.tile([70, 2, N_LOC], E5)
            emain = cpool.tile([P, K], F16)
            ecorr = cpool.tile([70, 2, K], E4)
            xrm = cpool.tile([P, NT, 68], F32)

            coff = cpool.tile([P, 2], F32)
            nc.vector.memset(coff[:, 0:1], -2.0)
            nc.vector.memset(coff[:, 1:2], 2046.0)
            nc.vector.memset(xrm[:, :, :], 0.0)
            for t in range(NT):
                nc.sync.dma_start(out=xrm[:, t, 0:D],
                                  in_=xrm_in[t * P:(t + 1) * P, :])
            nc.vector.memset(xrm[:, :, 64:65], 1.0)

            with tc.tile_pool(name="stage", bufs=1) as stpool:
                # ---- x tables ----
                SW = 4096
                for cidx in range(N_LOC // SW):
                    sl = slice(cidx * SW, (cidx + 1) * SW)
                    st = stpool.tile([P, SW], F32, tag="A")
                    nc.sync.dma_start(out=st[0:D, :], in_=xT_in[:, sl])
                    nc.sync.dma_start(out=st[D:P, :], in_=xT_in[:, sl])
                    x1 = stpool.tile([P, SW], F16, tag="H")
                    nc.vector.tensor_copy(out=x1[:, :], in_=st[:, :])
                    nc.vector.tensor_copy(out=xs[0:D, sl], in_=x1[0:D, :])
                    x1f = stpool.tile([P, SW], F32, tag="B")
                    nc.vector.tensor_copy(out=x1f[D:P, :], in_=x1[D:P, :])
                    nc.vector.tensor_sub(xs[D:P, sl], st[D:P, :], x1f[D:P, :])
                    xsc = stpool.tile([P, SW], F32, tag="C")
                    nc.vector.tensor_scalar_mul(xsc[0:D, :], st[0:D, :],
                                                1.0 / SCALE)
                    w0 = stpool.tile([P, SW], E5, tag="Q")
                    nc.vector.tensor_copy(out=w0[0:D, :], in_=xsc[0:D, :])
                    nc.vector.tensor_copy(out=w8[0:D, 0, sl], in_=w0[0:D, :])
                    w0f = stpool.tile([P, SW], F32, tag="B")
                    nc.vector.tensor_copy(out=w0f[0:D, :], in_=w0[0:D, :])
                    nc.vector.tensor_sub(xsc[0:D, :], xsc[0:D, :], w0f[0:D, :])
                    nc.vector.tensor_copy(out=w8[0:D, 1, sl], in_=xsc[0:D, :])
                # scale rows: w8[64+i,0,:] = 2^-w_i via iota+exp2, slot1 = 0
                nc.vector.memset(w8[D:70, 0, :], 1.0)
                nc.vector.memset(w8[D:70, 1, :], 0.0)
                iof = stpool.tile([P, 1], I32, tag="i1")
                nc.gpsimd.iota(iof[:, :], pattern=[[1, 1]], channel_multiplier=1)
                iff = stpool.tile([P, 1], F32, tag="i2")
                nc.vector.tensor_copy(out=iff[:, :], in_=iof[:, :])
                # u = -4*(p-64)-2 = -4p + 254  -> val = max(2^u, 2^-14)
                uexp = stpool.tile([P, 1], F32, tag="i3")
                nc.vector.tensor_scalar(uexp[:, :], iff[:, :], -4.0, 254.0,
                                        mybir.AluOpType.mult,
                                        mybir.AluOpType.add)
                vexp = stpool.tile([P, 1], F32, tag="i4")
                nc.scalar.activation(out=vexp[:, :], in_=uexp[:, :],
                                     func=mybir.ActivationFunctionType.Exp,
                                     scale=float(np.log(2.0)))
                nc.vector.tensor_scalar_max(vexp[:, :], vexp[:, :],
                                            float(2.0 ** -14))
                nc.vector.tensor_scalar_mul(w8[D:70, 0, :], w8[D:70, 0, :],
                                            vexp[D:70, 0:1])

                # ---- codebook tables ----
                ebt = stpool.tile([P, K], F32, tag="A")
                nc.sync.dma_start(out=ebt[0:D, :], in_=embT_in[:, :])
                e1 = stpool.tile([P, K], F16, tag="H")
                nc.vector.tensor_copy(out=e1[0:D, :], in_=ebt[0:D, :])
                e1f = stpool.tile([P, K], F32, tag="B")
                nc.vector.tensor_copy(out=e1f[0:D, :], in_=e1[0:D, :])
                e2s = stpool.tile([P, K], F32, tag="C")
                nc.vector.tensor_sub(e2s[0:D, :], ebt[0:D, :], e1f[0:D, :])
                nc.vector.tensor_scalar_mul(e2s[0:D, :], e2s[0:D, :], SCALE)
                for g in range(2):
                    for par in range(2):
                        blk = slice((2 * g + par) * KHH, (2 * g + par + 1) * KHH)
                        src = slice(g * 2048 + par, (g + 1) * 2048, 2)
                        nc.vector.tensor_copy(out=emain[0:D, blk],
                                              in_=e1[0:D, src])
                        nc.vector.tensor_copy(out=ecorr[0:D, 0, blk],
                                              in_=e2s[0:D, src])
                nc.sync.dma_start(out=emain[D:P, :], in_=emain[0:D, :])
                nc.vector.tensor_copy(out=ecorr[0:D, 1, :], in_=ecorr[0:D, 0, :])

                # ---- bias c ----
                negh = stpool.tile([D, P], F32, tag="n")
                nc.vector.memset(negh[:, :], -0.5)
                sq = stpool.tile([P, K], F32, tag="B")
                nc.vector.tensor_mul(sq[0:D, :], ebt[0:D, :], ebt[0:D, :])
                cb = stpool.tile([P, K], F32, tag="A")
                for q in range(4):
                    pc = pepool.tile([P, KQ], F32, tag="pse")
                    for h in range(2):
                        nc.tensor.matmul(
                            out=pc[:, h * 512:(h + 1) * 512], lhsT=negh[:, :],
                            rhs=sq[0:D, q * KQ + h * 512:q * KQ + (h + 1) * 512],
                            start=True, stop=True)
                    nc.scalar.copy(out=cb[:, q * KQ:(q + 1) * KQ], in_=pc[:, :])
                nc.sync.dma_start(out=embC[:, 0:D], in_=emb_in[:, :])
                nc.sync.dma_start(out=embC[:, 64:65], in_=cb[0:1, :])
                zpad = stpool.tile([P, 96], F32, tag="z")
                nc.vector.memset(zpad[:, :], 0.0)
                nc.sync.dma_start(out=embC[:, 65:68], in_=zpad[:, :])
                rem = stpool.tile([P, K], F32, tag="C")
                for g in range(2):
                    for par in range(2):
                        blk = slice((2 * g + par) * KHH, (2 * g + par + 1) * KHH)
                        src = slice(g * 2048 + par, (g + 1) * 2048, 2)
                        nc.vector.tensor_copy(out=rem[:, blk], in_=cb[:, src])
                for i, (a, w) in enumerate(zip(CHUNK_A, CHUNK_W)):
                    sc1 = stpool.tile([P, K], F32, tag="A")
                    nc.vector.tensor_scalar_mul(sc1[:, :], rem[:, :],
                                                float(2.0 ** a))
                    c8 = stpool.tile([P, K], E4, tag="Q")
                    nc.vector.tensor_copy(out=c8[:, :], in_=sc1[:, :])
                    sc2 = stpool.tile([P, K], F32, tag="A")
                    nc.vector.tensor_copy(out=sc2[:, :], in_=c8[:, :])
                    sc3 = stpool.tile([P, K], F32, tag="B")
                    nc.vector.tensor_scalar_mul(sc3[:, :], sc2[:, :],
                                                float(2.0 ** (-a)))
                    nc.vector.tensor_sub(rem[:, :], rem[:, :], sc3[:, :])
                    nc.vector.tensor_scalar_mul(sc3[:, :], sc2[:, :],
                                                float(2.0 ** (w - a)))
                    c8b = stpool.tile([P, K], E4, tag="Q")
                    nc.vector.tensor_copy(out=c8b[:, :], in_=sc3[:, :])
                    nc.sync.dma_start(out=ecorr[D + i:D + i + 1, 0, :],
                                      in_=c8b[0:1, :])
                c80 = stpool.tile([P, K], E4, tag="Q")
                nc.vector.memset(c80[:, :], 0.0)
                nc.sync.dma_start(out=ecorr[D:70, 1, :], in_=c80[0:6, :])

            # ---------------- main loop ----------------
            def tile_body(t):
                tsl = slice(t * P, (t + 1) * P)
                codes = mpool.tile([P, 2], F32, tag="codes")
                for g in range(2):
                    pse = pepool.tile([P, KQ], F32, tag="pse")
                    pso = popool.tile([P, KQ], F32, tag="pso")
                    so = sopool.tile([P, KQ], F32, tag="so")
                    for par, ps in ((0, pse), (1, pso)):
                        base = (2 * g + par) * KQ
                        for h in range(2):
                            mo = slice(base + h * 512, base + (h + 1) * 512)
                            nc.tensor.matmul(
                                out=ps[:, h * 512:(h + 1) * 512],
                                lhsT=xs[:, tsl], rhs=emain[:, mo],
                                start=True, stop=False)
                        for h in range(2):
                            mo = slice(base + h * 512, base + (h + 1) * 512)
                            nc.tensor.matmul(
                                out=ps[:, h * 512:(h + 1) * 512],
                                lhsT=w8[:, :, tsl], rhs=ecorr[:, :, mo],
                                start=False, stop=True,
                                perf_mode=mybir.MatmulPerfMode.DoubleRow,
                                skip_group_check=True)
                    nc.scalar.copy(out=so[:, :], in_=pso[:, :])
                    junk = jpool.tile([P, KQ], F32, tag="junk")
                    nc.vector._custom_dve(
                        argmax_op, out=junk[:, :], in0=pse[:, :], in1=so[:, :],
                        s1=2.0, accum_out=codes[:, g:g + 1])
                offs = mpool.tile([P, 2], F32, tag="offs")
                nc.gpsimd.tensor_tensor(out=offs[:, :], in0=codes[:, :],
                                        in1=coff[:, :], op=mybir.AluOpType.add)
                offi = mpool.tile([P, 2], I32, tag="offi")
                nc.vector.tensor_copy(out=offi[:, :], in_=offs[:, :])
                gath = gpool.tile([P, 2, 68], F32, tag="gath")
                nc.gpsimd.indirect_dma_start(
                    out=gath[:, 0, :], out_offset=None, in_=embC[:, :],
                    in_offset=bass.IndirectOffsetOnAxis(ap=offi[:, 0:1], axis=0),
                )
                nc.gpsimd.indirect_dma_start(
                    out=gath[:, 1, :], out_offset=None, in_=embC[:, :],
                    in_offset=bass.IndirectOffsetOnAxis(ap=offi[:, 1:2], axis=0),
                )
                xr2 = gpool.tile([P, 2, 68], F32, tag="xr2")
                nc.scalar.copy(out=xr2[:, 0, :], in_=xrm[:, t, :])
                nc.scalar.copy(out=xr2[:, 1, :], in_=xrm[:, t, :])
                csum = gpool.tile([P, 136], F32, tag="csum")
                nc.vector._custom_dve(
                    dot_op, out=csum[:, :], in0=gath[:, :, :], in1=xr2[:, :, :])
                # msk = (v135 > 2*v67)
                tw = mpool.tile([P, 1], F32, tag="tw")
                nc.scalar.mul(tw[:, :], csum[:, 67:68], 2.0)
                msk = mpool.tile([P, 1], F32, tag="msk")
                nc.vector.tensor_tensor(out=msk[:, :], in0=csum[:, 135:136],
                                        in1=tw[:, :], op=mybir.AluOpType.is_gt)
                # rf = msk*g1 + (1-msk)*g0  (exact when msk in {0,1})
                imsk = mpool.tile([P, 1], F32, tag="imsk")
                nc.scalar.activation(imsk[:, :], msk[:, :],
                                     mybir.ActivationFunctionType.Copy,
                                     bias=1.0, scale=-1.0)
                dif = gpool.tile([P, D], F32, tag="dif")
                nc.scalar.mul(dif[:, :], gath[:, 1, 0:D], msk[:, 0:1])
                rf = gpool.tile([P, D], F32, tag="rf")
                nc.scalar.mul(rf[:, :], gath[:, 0, 0:D], imsk[:, 0:1])
                nc.gpsimd.tensor_tensor(out=rf[:, :], in0=rf[:, :],
                                        in1=dif[:, :], op=mybir.AluOpType.add)
                nc.sync.dma_start(out=q_out[tsl, :], in_=rf[:, :])

            if r_iters == 1:
                for t in range(NT):
                    tile_body(t)
            else:
                with tc.For_i(0, r_iters, 1):
                    for t in range(NT):
                        tile_body(t)

    nc.compile()
    return nc


def make_in_maps(x: np.ndarray, embed: np.ndarray):
    x = np.ascontiguousarray(x, dtype=np.float32)
    embed = np.ascontiguousarray(embed, dtype=np.float32)
    embT = np.ascontiguousarray(embed.T)
    return [
        {
            "xT": np.ascontiguousarray(x[c * N_LOC:(c + 1) * N_LOC].T),
            "embT": embT,
            "embed": embed,
            "xrm": x[c * N_LOC:(c + 1) * N_LOC],
        }
        for c in range(N_CORES)
    ]


_CACHED_NC = None


def kernel(x: np.ndarray, embed: np.ndarray) -> np.ndarray:
    global _CACHED_NC
    assert x.shape == (N_FULL, D) and embed.shape == (K, D)
    if _CACHED_NC is None:
        _CACHED_NC = build()
    res = run_bass_kernel_spmd(
        _CACHED_NC, make_in_maps(x, embed), core_ids=list(range(N_CORES))
    )
    return np.concatenate([r["q"] for r in res.results], axis=0)
